# revision 13
# baseline (speedup 1.0000x reference)
"""Causal attention kernel for Trainium2 (Bass/Tile), SPMD over 8 NeuronCores.

Problem: B=16, N=2048, D=256 fp32 causal attention with padding mask.
Sharding: batch dim across 8 cores (2 batches per core); attention is
batch-independent so no collectives are needed.

Per-core algorithm (S^T orientation: k on partitions, q on free axis):
  S^T = K @ Q^T computed chunkwise as (K^T chunk).T @ Q^T  [fp32r matmuls]
  P^T = exp(scale * S^T + pad_bias)   [ScalarE, pad bias is per-partition]
  [O | rowsum] = P @ [V | 1]          [ones-column gives softmax denominators]
  O = O * (1/rowsum)
Q^T and K^T are built with PE transposes of natural-layout 128x128 chunks.
"""

import numpy as np

import concourse.bass as bass
from concourse import bacc
import concourse.mybir as mybir
from concourse import tile
from concourse.masks import make_identity
from concourse.bass_utils import run_bass_kernel_spmd

F32 = mybir.dt.float32
F32R = mybir.dt.float32r
I32 = mybir.dt.int32

N_CORES = 8
B_FULL, N_SEQ, D_MODEL = 16, 2048, 256
B_LOCAL = B_FULL // N_CORES

NEG = -1e30
P = 128


def build_attention_nc(B=B_LOCAL, N=N_SEQ, D=D_MODEL, mm_dtype=F32R):
    nc = bacc.Bacc(num_swdge_queues=4)
    NT = N // P            # number of 128-row tiles along sequence
    DC = D // P            # number of 128-wide d chunks
    NB = NT // 2           # q blocks of 256 (2 q-tiles each)
    scale = 1.0 / float(np.sqrt(D))

    q_d = nc.declare_dram_parameter("q", [B, N, D], F32, isOutput=False)
    k_d = nc.declare_dram_parameter("k", [B, N, D], F32, isOutput=False)
    v_d = nc.declare_dram_parameter("v", [B, N, D], F32, isOutput=False)
    pm_d = nc.declare_dram_parameter("pm", [B, N], I32, isOutput=False)
    o_d = nc.declare_dram_parameter("o", [B, N, D], F32, isOutput=True)

    with tile.TileContext(nc) as tc:
        with (
            tc.tile_pool(name="consts", bufs=1) as consts,
            tc.tile_pool(name="big", bufs=2) as big,
            tc.tile_pool(name="natp", bufs=4) as natp,
            tc.tile_pool(name="ptp", bufs=4) as ptp,
            tc.tile_pool(name="smallp", bufs=4) as smallp,
            tc.tile_pool(name="ps_tp", bufs=2, space="PSUM") as ps_tp,
            tc.tile_pool(name="ps_sp", bufs=2, space="PSUM") as ps_sp,
            tc.tile_pool(name="ps_op", bufs=4, space="PSUM") as ps_op,
        ):
            identity = consts.tile([P, P], F32)
            make_identity(nc, identity)
            # Additive causal mask for the diagonal 128x128 chunk of S^T:
            # element [k_local, q_local] valid iff k <= q, i.e. keep where
            # (q - k) >= 0, else fill with NEG.
            dmask = consts.tile([P, P], F32)
            nc.gpsimd.memset(dmask, 0.0)
            nc.gpsimd.affine_select(
                out=dmask,
                in_=dmask,
                compare_op=mybir.AluOpType.is_ge,
                fill=NEG,
                base=0,
                pattern=[[1, P]],
                channel_multiplier=-1,
            )

            for b in range(B):
                # ---- per-batch loads ----
                # fp32r tiles: the PE requires fp32r matmul operands to be
                # written (rounded) by their producer, so these are produced
                # by DVE copies / the exp activation, never raw DMA.
                kT = big.tile([P, DC, N], mm_dtype, tag="kT")
                qT = big.tile([P, DC, N], mm_dtype, tag="qT")
                vx = big.tile([P, NT, D + 4], mm_dtype, tag="vx")
                ostg = big.tile([P, NT, D], F32, tag="ostg")
                pbias = big.tile([P, NT], F32, tag="pbias")

                G = min(8, NT)  # chunks per DMA group
                for g0 in range(0, NT, G):
                    vg = natp.tile([P, G, D], F32, tag="nat")
                    nc.gpsimd.dma_start(
                        out=vg,
                        in_=v_d[b].rearrange("(c p) d -> p c d", p=P)[
                            :, g0 : g0 + G, :
                        ],
                    )
                    nc.vector.tensor_copy(vx[:, g0 : g0 + G, 0:D], vg)
                # col D = ones (softmax denominator trick); D+1..D+3 zero pad
                # so the PV moving operand width (260) is 16B-aligned.
                ones_t = smallp.tile([P, NT, 4], F32, tag="ones")
                nc.gpsimd.memset(ones_t, 0.0)
                nc.gpsimd.memset(ones_t[:, :, 0], 1.0)
                nc.vector.tensor_copy(vx[:, :, D : D + 4], ones_t)

                pmi = smallp.tile([P, NT], I32, tag="pmi")
                nc.gpsimd.dma_start(
                    out=pmi, in_=pm_d[b].rearrange("(c p) -> p c", p=P)
                )
                pmf = smallp.tile([P, NT], F32, tag="pmf")
                nc.vector.tensor_copy(pmf, pmi)
                # bias = (min(pm,1) - 1) * 1e30  -> 0 where pm!=0, -1e30 where pm==0
                tmp = smallp.tile([P, NT], F32, tag="tmp")
                nc.vector.tensor_scalar(
                    out=tmp,
                    in0=pmf,
                    scalar1=1.0,
                    scalar2=None,
                    op0=mybir.AluOpType.min,
                )
                nc.vector.tensor_scalar(
                    out=pbias,
                    in0=tmp,
                    scalar1=-1.0,
                    scalar2=-NEG,
                    op0=mybir.AluOpType.add,
                    op1=mybir.AluOpType.mult,
                )

                # ---- transposes: build qT, kT (d on partitions) ----
                for src_d, dst in ((q_d, qT), (k_d, kT)):
                    for g0 in range(0, NT, G):
                        nat = natp.tile([P, G, D], F32, tag="nat")
                        nc.gpsimd.dma_start(
                            out=nat,
                            in_=src_d[b].rearrange("(c p) d -> p c d", p=P)[
                                :, g0 : g0 + G, :
                            ],
                        )
                        for i in range(G):
                            nch = g0 + i
                            for dc in range(DC):
                                pst = ps_tp.tile([P, P], F32, tag="pst")
                                nc.tensor.transpose(
                                    pst, nat[:, i, dc * P : (dc + 1) * P], identity
                                )
                                nc.vector.tensor_copy(
                                    dst[:, dc, nch * P : (nch + 1) * P], pst
                                )

                # ---- main attention loop over q blocks of 256 ----
                for qb in range(NB):
                    t0, t1 = 2 * qb, 2 * qb + 1
                    po0 = ps_op.tile([P, D + 4], F32, tag="po")
                    po1 = ps_op.tile([P, D + 4], F32, tag="po")
                    po = (po0, po1)
                    for j in range(t1 + 1):
                        ss = ps_sp.tile([P, 256], F32, tag="ss")
                        for dc in range(DC):
                            nc.tensor.matmul(
                                ss,
                                kT[:, dc, j * P : (j + 1) * P],
                                qT[:, dc, qb * 256 : qb * 256 + 256],
                                start=(dc == 0),
                                stop=(dc == DC - 1),
                            )
                        if j == t0:
                            nc.vector.tensor_add(ss[:, 0:P], ss[:, 0:P], dmask)
                        if j == t1:
                            nc.vector.tensor_add(ss[:, P:256], ss[:, P:256], dmask)
                        ls = 0 if j <= t0 else P
                        pt = ptp.tile([P, 256], mm_dtype, tag="pt")
                        nc.scalar.activation(
                            pt[:, ls:256],
                            ss[:, ls:256],
                            mybir.ActivationFunctionType.Exp,
                            bias=pbias[:, j : j + 1],
                            scale=scale,
                        )
                        for ti, t in ((0, t0), (1, t1)):
                            if j <= t:
                                nc.tensor.matmul(
                                    po[ti],
                                    pt[:, ti * P : (ti + 1) * P],
                                    vx[:, j, 0 : D + 4],
                                    start=(j == 0),
                                    stop=(j == t),
                                )
                    for ti, t in ((0, t0), (1, t1)):
                        rec = smallp.tile([P, 1], F32, tag="rec")
                        nc.vector.reciprocal(rec, po[ti][:, D : D + 1])
                        nc.vector.tensor_scalar_mul(
                            ostg[:, t, :], po[ti][:, 0:D], rec
                        )

                nc.gpsimd.dma_start(
                    out=o_d[b].rearrange("(c p) d -> p c d", p=P), in_=ostg
                )

    nc.finalize()
    return nc


_NC_CACHE = {}


def _get_nc():
    key = (B_LOCAL, N_SEQ, D_MODEL)
    if key not in _NC_CACHE:
        _NC_CACHE[key] = build_attention_nc()
    return _NC_CACHE[key]


def kernel(Q, K, V, padding_mask):
    Q = np.ascontiguousarray(np.asarray(Q), dtype=np.float32)
    K = np.ascontiguousarray(np.asarray(K), dtype=np.float32)
    V = np.ascontiguousarray(np.asarray(V), dtype=np.float32)
    pm = np.ascontiguousarray(np.asarray(padding_mask), dtype=np.int32)

    nc = _get_nc()
    in_maps = []
    for c in range(N_CORES):
        s = slice(c * B_LOCAL, (c + 1) * B_LOCAL)
        in_maps.append({"q": Q[s], "k": K[s], "v": V[s], "pm": pm[s]})
    res = run_bass_kernel_spmd(nc, in_maps, list(range(N_CORES)))
    out = np.concatenate([res.results[c]["o"] for c in range(N_CORES)], axis=0)
    return out.astype(np.float32)
